# revision 43
# baseline (speedup 1.0000x reference)
"""Trainium2 Bass kernel for EpisodicMemory (DMN episodic memory module).

Full shapes: facts (128,256,512), questions/prevM (128,1,512), output (128,1,512).
Sharding: data-parallel over batch, 16 batches per core x 8 cores, weights
replicated. Everything on-chip (no DRAM scratch): activations are kept
feature-on-partition ("transposed") so matmuls contract over the partition
dim and pointwise ops run 128 lanes wide.

Per-core pipeline:
  P1  transpose facts to fT via PE transposes (2-batch groups)
  P2  interaction features zT (bf16) -> z1 MLP (tanh) -> z2 logits
  P3  pre_r = Wr@f + (br+bur), pre_h = W@f + bw   (stored bf16 in SBUF)
  P4  softmax over S -> G; broadcast G to all partitions (DRAM bounce)
  P5  GRU scan over S=256 steps in delta form: psr/psu live in PSUM for the
      whole scan and accumulate W^T @ gd_s (gd = per-step state delta, bf16).
      Weights bf16 (1 cycle/row vs 4 for f32r). bu folded into the psu PSUM
      init via K=1 matmuls. psr-gate matmuls issue first so the sigmoid path
      overlaps the psu-gate matmuls.
  P6  next_mem = relu([prevM C q] @ nm_w + nm_b) via C-stationary matmuls
"""

from contextlib import ExitStack

import numpy as np

import concourse.bass as bass
import concourse.tile as tile
from concourse import bacc, masks, mybir
from concourse.bass_utils import run_bass_kernel_spmd

F32 = mybir.dt.float32
F32R = mybir.dt.float32r
BF16 = mybir.dt.bfloat16
FP8 = mybir.dt.float8e4
AF = mybir.ActivationFunctionType
ALU = mybir.AluOpType
DR = mybir.MatmulPerfMode.DoubleRow
Z1SC = 16.0  # fp8 scale for z1_w (values ~N(0, 0.02) -> normal e4m3 range)

B, S, H = 128, 256, 512
N_CORES = 8
B_LOC = B // N_CORES  # 16


def build_nc(b_loc=B_LOC, s_len=S):
    """Build the per-core Bass program (SPMD: same program, sharded data)."""
    h = H
    nc = bacc.Bacc(
        "TRN2", target_bir_lowering=False, debug=False, num_devices=N_CORES
    )

    io = {}
    io["facts"] = nc.dram_tensor("facts", [b_loc, s_len, h], F32, kind="ExternalInput")
    io["questions"] = nc.dram_tensor("questions", [b_loc, 1, h], F32, kind="ExternalInput")
    io["prevM"] = nc.dram_tensor("prevM", [b_loc, 1, h], F32, kind="ExternalInput")
    io["z1_w"] = nc.dram_tensor("z1_w", [4 * h, h], F32, kind="ExternalInput")
    io["z1_b"] = nc.dram_tensor("z1_b", [h], F32, kind="ExternalInput")
    io["z2_w"] = nc.dram_tensor("z2_w", [h, 1], F32, kind="ExternalInput")
    for nm in ["Wr", "Ur", "W", "U"]:
        io[nm] = nc.dram_tensor(nm, [h, h], F32, kind="ExternalInput")
    for nm in ["br", "bur", "bw", "bu"]:
        io[nm] = nc.dram_tensor(nm, [h], F32, kind="ExternalInput")
    io["nm_w"] = nc.dram_tensor("nm_w", [3 * h, h], F32, kind="ExternalInput")
    io["nm_b"] = nc.dram_tensor("nm_b", [h], F32, kind="ExternalInput")
    io["out"] = nc.dram_tensor("out", [b_loc, 1, h], F32, kind="ExternalOutput")
    io["g_bounce"] = nc.dram_tensor("g_bounce", [s_len + s_len // 8, b_loc], F32)
    io["logit_dram"] = nc.dram_tensor("logit_dram", [b_loc, s_len], F32)

    with tile.TileContext(nc) as tc:
        _body(tc, io, b_loc, s_len, h)
    nc.compile()
    return nc


def _body(tc, io, b_loc, s_len, h):
    nc = tc.nc
    hc = h // 128          # 4 h-chunks
    zc = 4 * hc            # 16 chunks of the 4H interaction dim
    gb = 2                 # batches per group (matmul moving dim = gb*s_len)
    ng = b_loc // gb
    sc_ = s_len // 128

    facts, questions, prevM = io["facts"], io["questions"], io["prevM"]

    with ExitStack() as ctx:
        # ---------------- resident pools ----------------
        wpool = ctx.enter_context(tc.tile_pool(name="wres", bufs=1))
        prepool = ctx.enter_context(tc.tile_pool(name="prepool", bufs=1))
        smallpool = ctx.enter_context(tc.tile_pool(name="small", bufs=1))
        pfpool = ctx.enter_context(tc.tile_pool(name="pf", bufs=1))

        # prefetch the first 2-batch group of facts ahead of the weight DMAs
        # so the PE can start transposing ~40us earlier
        fpre = pfpool.tile([128, gb * sc_, h], F32, tag="fpre")
        for bp in range(gb):
            for sh in range(sc_):
                nc.sync.dma_start(
                    fpre[:, bp * sc_ + sh, :],
                    facts[bp, sh * 128:(sh + 1) * 128, :],
                )

        # scan gate weights [Ur | U]: k-chunk c at cols [c*2h, (c+1)*2h)
        # (DMAs are emitted in P5 so they don't delay the P1/P2 startup)
        wcomb = wpool.tile([128, hc * 2 * h], F32R, tag="wcomb")

        # small constants: (128, hc) with col = h-chunk
        def load_cvec(nm):
            t = smallpool.tile([128, hc], F32, tag=f"cv_{nm}")
            nc.sync.dma_start(t[:, :], io[nm].rearrange("(c p) -> p c", p=128))
            return t

        z1b4 = load_cvec("z1_b")
        br4 = load_cvec("br")
        bur4 = load_cvec("bur")
        bw4 = load_cvec("bw")
        z2c = smallpool.tile([128, hc], BF16, tag="z2c")
        z2stg = smallpool.tile([128, hc], F32, tag="z2stg")
        nc.sync.dma_start(
            z2stg[:, :], io["z2_w"].rearrange("(c p) o -> p (c o)", p=128)
        )
        nc.vector.tensor_copy(z2c[:, :], z2stg[:, :])
        brc4 = smallpool.tile([128, hc], F32, tag="brc4")  # br + bur
        nc.vector.tensor_copy(brc4[:, :], br4[:, :])
        nc.vector.tensor_add(brc4[:, :], brc4[:, :], bur4[:, :])

        # bu as a row [1, h] (bf16) + ones row for psu PSUM bias init
        bu_stg = smallpool.tile([1, h], F32, tag="bu_stg")
        nc.sync.dma_start(bu_stg[:, :], io["bu"][None, :])
        bu_row = smallpool.tile([1, h], BF16, tag="bu_row")
        nc.vector.tensor_copy(bu_row[:, :], bu_stg[:, :])
        onesb_stg = smallpool.tile([1, b_loc], F32, tag="onesb_stg")
        nc.vector.memset(onesb_stg[:, :], 1.0)
        onesb = smallpool.tile([1, b_loc], BF16, tag="onesb")
        nc.vector.tensor_copy(onesb[:, :], onesb_stg[:, :])

        # questions / prevM transposed, b-major free layout (128, b_loc, hc)
        # so the gather merges into one DMA descriptor each; the dma_start
        # calls are emitted after the z1 weight load (they'd block the sync
        # queue ~6us each otherwise)
        qT = smallpool.tile([128, b_loc, hc], F32R, tag="qT")
        mT = smallpool.tile([128, b_loc, hc], F32R, tag="mT")
        nqT = smallpool.tile([128, b_loc, hc], F32, tag="nqT")
        nmT = smallpool.tile([128, b_loc, hc], F32, tag="nmT")

        ones_row = smallpool.tile([1, b_loc], F32R, tag="ones_row")
        nc.vector.tensor_copy(ones_row[:, :], onesb_stg[:, :])
        nmb_row = smallpool.tile([1, h], F32R, tag="nmb_row")
        nc.sync.dma_start(nmb_row[:, :], io["nm_b"][None, :].bitcast(F32R))

        ident = smallpool.tile([128, 128], F32, tag="ident")
        masks.make_identity(nc, ident[:, :])

        # pre-activations resident through the scan: [p, gate, s, m, b] bf16
        # (s-major so the per-step slice [m, b] is contiguous)
        pre_sb = prepool.tile([128, 2, s_len, hc, b_loc], BF16, tag="pre_sb")
        logit = smallpool.tile([b_loc, s_len], F32, tag="logit")

        # ============ phases P1..P3 (per 2-batch group) ============
        with (
            tc.tile_pool(name="phw", bufs=1) as phw,
            tc.tile_pool(name="ph", bufs=2) as ph,
            tc.tile_pool(name="zpool", bufs=3) as zp,
            tc.tile_pool(name="ghpool", bufs=1) as ghpool,
            tc.tile_pool(name="tps", bufs=2, space="PSUM") as tps,
            tc.tile_pool(name="ghps", bufs=1, space="PSUM") as ghps,
            tc.tile_pool(name="lgps", bufs=1, space="PSUM") as lgps,
        ):
            # z1 weights in fp8e4 (scaled by Z1SC; staged through f32),
            # laid out [128, k-tile, h] for DoubleRow matmuls. One big DMA +
            # one big cast — a chunked DMA/cast pipeline here stalls the
            # whole P2 startup on staging-buffer reuse.
            # z1 weights: 4 chunked DMA+cast stages so the first DoubleRow
            # matmuls (k-tiles 0,1) can start ~10us in; q/m gathers (slow
            # 4B-element DMAs) interleave after the first chunk
            z1stg = phw.tile([128, zc, h], F32, tag="z1stg")
            z1w = phw.tile([128, zc, h], FP8, tag="z1w")
            zw_src = io["z1_w"].rearrange("(k p) h -> p k h", p=128)
            for ch in range(4):
                kk = slice(4 * ch, 4 * ch + 4)
                nc.sync.dma_start(z1stg[:, kk, :], zw_src[:, kk, :])
                nc.vector.tensor_scalar_mul(
                    z1w[:, kk, :], z1stg[:, kk, :], Z1SC
                )
                if ch == 0:
                    nc.sync.dma_start(
                        qT[:, :, :],
                        questions[:, 0, :].rearrange(
                            "b (c p) -> p b c", p=128).bitcast(F32R),
                    )
                    nc.sync.dma_start(
                        mT[:, :, :],
                        prevM[:, 0, :].rearrange(
                            "b (c p) -> p b c", p=128).bitcast(F32R),
                    )
                    nc.vector.tensor_scalar_mul(
                        nqT[:, :, :], qT[:, :, :].bitcast(F32), -1.0
                    )
                    nc.vector.tensor_scalar_mul(
                        nmT[:, :, :], mT[:, :, :].bitcast(F32), -1.0
                    )
            # row-sums of Wr / W (the reference's einsum "bsh,hk->bsh"
            # multiplies facts elementwise by these row-sums)
            rs4 = phw.tile([128, hc, 2], F32, tag="rs4")
            for gate, wname in enumerate(["Wr", "W"]):
                wstg = phw.tile([128, hc, h], F32, tag=f"wstg{gate}")
                nc.sync.dma_start(
                    wstg[:, :, :],
                    io[wname].rearrange("(c p) h -> p c h", p=128),
                )
                for c in range(hc):
                    nc.vector.tensor_reduce(
                        rs4[:, c, gate:gate + 1], wstg[:, c, :],
                        mybir.AxisListType.X, ALU.add,
                    )

            for g in range(ng):
                # ---- P1: load + transpose facts ----
                fT2 = ph.tile([128, hc, gb * s_len], BF16, tag="fT2")
                for bp in range(gb):
                    bi = g * gb + bp
                    for sh in range(sc_):
                        if g == 0:
                            fnat = fpre[:, bp * sc_ + sh, :]
                        else:
                            fnat = ph.tile([128, h], F32, tag="fnat")
                            nc.sync.dma_start(
                                fnat[:, :], facts[bi, sh * 128:(sh + 1) * 128, :]
                            )
                        tp = tps.tile([128, hc, 128], F32, tag="tpsum")
                        for c in range(hc):
                            nc.tensor.transpose(
                                tp[:, c, :], fnat[:, c * 128:(c + 1) * 128],
                                ident[:, :],
                            )
                        nc.vector.tensor_copy(
                            fT2[:, :, bp * s_len + sh * 128:
                                bp * s_len + (sh + 1) * 128],
                            tp[:, :, :],
                        )

                # ---- P2: interaction features (fp8) + z1 MLP (DoubleRow)
                #      + z2 logits ----
                ghp = [ghps.tile([128, gb * s_len], F32, name=f"ghp{m}",
                                 tag=f"ghp{m}") for m in range(hc)]
                for j in range(zc // 2):
                    zk2 = zp.tile([128, 2, gb * s_len], FP8, tag="zk")
                    for half in range(2):
                        k = 2 * j + half
                        kind, c = divmod(k, hc)  # 0:f*q 1:f*m 2:|f-q| 3:|f-m|
                        for bp in range(gb):
                            bi = g * gb + bp
                            dst = zk2[:, half, bp * s_len:(bp + 1) * s_len]
                            src = fT2[:, c, bp * s_len:(bp + 1) * s_len]
                            if kind == 0:
                                nc.vector.tensor_scalar_mul(
                                    dst, src, qT[:, bi, c:c + 1].bitcast(F32))
                            elif kind == 1:
                                nc.vector.tensor_scalar_mul(
                                    dst, src, mT[:, bi, c:c + 1].bitcast(F32))
                            elif kind == 2:
                                nc.scalar.activation(dst, src, AF.Abs,
                                                     bias=nqT[:, bi, c:c + 1])
                            else:
                                nc.scalar.activation(dst, src, AF.Abs,
                                                     bias=nmT[:, bi, c:c + 1])
                    for m in range(hc):
                        nc.tensor.matmul(
                            ghp[m][:, :],
                            z1w[:, 2 * j:2 * j + 2, m * 128:(m + 1) * 128],
                            zk2[:, :, :],
                            start=(j == 0),
                            stop=(j == zc // 2 - 1),
                            perf_mode=DR,
                        )
                ghT = ph.tile([128, hc, gb * s_len], BF16, tag="ghT")
                for m in range(hc):
                    nc.scalar.activation(
                        ghT[:, m, :], ghp[m][:, :], AF.Tanh,
                        bias=z1b4[:, m:m + 1], scale=1.0 / Z1SC,
                    )
                lgp = lgps.tile([1, gb * s_len], F32, tag="lgp")
                for m in range(hc):
                    nc.tensor.matmul(
                        lgp[:, :], z2c[:, m:m + 1], ghT[:, m, :],
                        start=(m == 0), stop=(m == hc - 1),
                    )
                lstage = ph.tile([1, gb * s_len], F32, tag="lstage")
                nc.vector.tensor_copy(lstage[:, :], lgp[:, :])
                nc.sync.dma_start(
                    io["logit_dram"][None, g * gb:(g + 1) * gb, :],
                    lstage[:, :].rearrange("o (b s) -> o b s", b=gb),
                )

                # ---- P3: pre_r / pre_h = facts * rowsum(W) + bias ----
                for gate in range(2):
                    bias4 = brc4 if gate == 0 else bw4
                    for m in range(hc):
                        nc.vector.tensor_scalar(
                            pre_sb[:, gate, :, m, g * gb:(g + 1) * gb],
                            fT2[:, m, :].rearrange("p (b s) -> p s b", b=gb),
                            rs4[:, m, gate:gate + 1],
                            bias4[:, m:m + 1],
                            ALU.mult,
                            ALU.add,
                        )

        # ============ P4: softmax over S + blend-weight broadcast ============
        # The scan processes blocks of BLK steps with C frozen per block; the
        # within-block state update telescopes exactly:
        #   D_BLK = sum_t w_t * e_t,  w_t = g_t * prod_{u>t} (1 - g_u)
        # Compute w in (b, s) layout, then broadcast to all partitions.
        BLK = 8
        abw = smallpool.tile([128, s_len + s_len // BLK, 1, b_loc], F32, tag="abw")
        abc = abw[:, :s_len, :, :]     # w_t broadcast
        wsc = abw[:, s_len:, :, :]     # per-block sum_t w_t broadcast
        with tc.tile_pool(name="smax", bufs=1) as sp:
            nc.sync.dma_start(logit[:, :], io["logit_dram"][:, :])
            negmax = sp.tile([b_loc, 1], F32, tag="negmax")
            nc.vector.tensor_reduce(
                negmax[:, :], logit[:, :], mybir.AxisListType.X, ALU.max, negate=True
            )
            esum = sp.tile([b_loc, 1], F32, tag="esum")
            gexp = sp.tile([b_loc, s_len], F32, tag="gexp")
            nc.scalar.activation(
                gexp[:, :], logit[:, :], AF.Exp, bias=negmax[:, :],
                accum_out=esum[:, :],
            )
            inv = sp.tile([b_loc, 1], F32, tag="inv")
            nc.vector.reciprocal(inv[:, :], esum[:, :])
            gmat = sp.tile([b_loc, s_len], F32, tag="gmat")
            nc.vector.tensor_scalar_mul(gmat[:, :], gexp[:, :], inv[:, :])

            # blend weights w_t = g_t * prod_{u>t in blk} (1-g_u) via suffix
            # products within each BLK-sized block
            om = sp.tile([b_loc, s_len], F32, tag="om")
            nc.vector.tensor_scalar(
                om[:, :], gmat[:, :], -1.0, 1.0, ALU.mult, ALU.add
            )
            wmat = sp.tile([b_loc, s_len], F32, tag="wmat")
            nbk = s_len // BLK
            nc.vector.tensor_copy(wmat[:, BLK - 1::BLK], gmat[:, BLK - 1::BLK])
            suf = sp.tile([b_loc, nbk], F32, tag="suf_a")
            nc.vector.tensor_copy(suf[:, :], om[:, BLK - 1::BLK])
            for t in range(BLK - 2, -1, -1):
                nc.vector.tensor_tensor(
                    wmat[:, t::BLK], gmat[:, t::BLK], suf[:, :], ALU.mult
                )
                if t > 0:
                    suf2 = sp.tile(
                        [b_loc, nbk], F32,
                        tag="suf_b" if (BLK - t) % 2 == 0 else "suf_a",
                    )
                    nc.vector.tensor_tensor(
                        suf2[:, :], om[:, t::BLK], suf[:, :], ALU.mult
                    )
                    suf = suf2
            # per-block sum of the w_t (pairwise reduce)
            cur = wmat
            width = s_len
            lvl = 0
            while width > nbk:
                nxt = sp.tile([b_loc, width // 2], F32, tag=f"ws{lvl}")
                nc.vector.tensor_tensor(
                    nxt[:, :], cur[:, 0::2], cur[:, 1::2], ALU.add
                )
                cur = nxt
                width //= 2
                lvl += 1

            # broadcast w + wsum to all partitions through a DRAM bounce
            nc.sync.dma_start(
                io["g_bounce"][:s_len, :].rearrange("s b -> b s"), wmat[:, :]
            )
            nc.sync.dma_start(
                io["g_bounce"][s_len:, :].rearrange("s b -> b s"), cur[:, :]
            )
            nc.sync.dma_start(
                abw[:, :, 0, :],
                io["g_bounce"][None, :, :].to_broadcast(
                    [128, s_len + s_len // BLK, b_loc]
                ),
            )

        # ============ P5: GRU scan (blocked steps, delta-PSUM form) ============
        # Steps are processed in blocks of BLK. The gate pre-activations for
        # block k use the state snapshot taken after block k-1's matmuls,
        # which itself lags one block (two-block-stale gates, <= 2*BLK-1
        # steps; g ~ 1/256 keeps the error ~1e-3 relative). This decouples
        # the PE pass and the PSUM->SBUF snapshot from the serial DVE/ACT
        # chain: they hide inside the sigmoid/tanh gaps. The within-block
        # state update is EXACT via the telescoped form
        #   D = sum_t w_t h_t - (sum_t w_t) C_s0.
        # psr/psu live in PSUM the whole scan and accumulate W^T @ D once per
        # block.
        n_blk = s_len // BLK
        with (
            tc.tile_pool(name="scw", bufs=1) as scw,
            tc.tile_pool(name="scan_sb", bufs=2) as scp,
            tc.tile_pool(name="scan_ps", bufs=1, space="PSUM") as sps,
            tc.tile_pool(name="out_ps", bufs=1, space="PSUM") as ops,
        ):
            # load + cast the scan weights to bf16 (1 cycle/row on the PE vs
            # 4 for f32r at narrow moving); DMAs deferred to here so the
            # P1/P2 startup isn't delayed
            for c in range(hc):
                nc.sync.dma_start(
                    wcomb[:, c * 2 * h: c * 2 * h + h],
                    io["Ur"][c * 128:(c + 1) * 128, :].bitcast(F32R),
                )
                nc.sync.dma_start(
                    wcomb[:, c * 2 * h + h: (c + 1) * 2 * h],
                    io["U"][c * 128:(c + 1) * 128, :].bitcast(F32R),
                )
            wb = scw.tile([128, hc * 2 * h], BF16, tag="wb")
            nc.vector.tensor_copy(wb[:, :], wcomb[:, :].bitcast(F32))

            # final-layer weights (loaded while the scan runs)
            nmw = scw.tile([128, 3 * hc * h], F32R, tag="nmw")
            for j in range(3 * hc):
                nc.sync.dma_start(
                    nmw[:, j * h:(j + 1) * h],
                    io["nm_w"][j * 128:(j + 1) * 128, :].bitcast(F32R),
                )

            # persistent PSUM accumulators: psr/psu = W^T @ C_{2k} (+ bu)
            psr = sps.tile([128, 1, hc, b_loc], F32, tag="psr")
            psu = sps.tile([128, 1, hc, b_loc], F32, tag="psu")
            # psu init = bu (broadcast over batch) via K=1 matmuls; psr gets
            # start=True on its first real matmul (pair k=1)
            for m in range(hc):
                nc.tensor.matmul(
                    psu[:, 0, m, :], bu_row[:, m * 128:(m + 1) * 128],
                    onesb[:, :], start=True, stop=False, skip_group_check=True,
                )

            ct = None     # C_{s0} (f32), set by the first tail
            wc = None     # (sum w)*C snapshot for the gsum of the next block
            bck = [128, BLK, hc, b_loc]
            bc1 = [128, 1, hc, b_loc]

            # bu broadcast for the first two blocks' gate math
            bu4 = load_cvec("bu")
            bu_bc = scw.tile(bc1, F32, tag="bu_bc")
            ones_pb = scw.tile([128, b_loc], F32, tag="ones_pb")
            nc.vector.memset(ones_pb[:, :], 1.0)
            for c in range(hc):
                nc.vector.tensor_scalar_mul(
                    bu_bc[:, 0, c, :], ones_pb[:, :], bu4[:, c:c + 1]
                )

            def chain(k, psrS, psuS):
                """gate math for block k using (stale) state snapshots;
                emits the ct/wc updates of block k-1 into the sigmoid gap."""
                s0 = BLK * k
                if psrS is None:
                    rtp = scp.tile(bck, BF16, tag="rtp")
                    nc.scalar.activation(
                        rtp[:, :, :, :], pre_sb[:, 0, s0:s0 + BLK, :, :],
                        AF.Sigmoid,
                    )
                else:
                    sgp = scp.tile(bck, BF16, tag="sgp")
                    nc.vector.tensor_tensor(
                        sgp[:, :, :, :],
                        psrS[:, 0:1, :, :].to_broadcast(bck),
                        pre_sb[:, 0, s0:s0 + BLK, :, :], ALU.add,
                    )
                    rtp = scp.tile(bck, BF16, tag="rtp")
                    nc.scalar.activation(
                        rtp[:, :, :, :], sgp[:, :, :, :], AF.Sigmoid
                    )
                # ct/wc updates of the previous block run during the sigmoid
                emit_state_update(k - 1)
                ut2p = scp.tile(bck, BF16, tag="ut2p")
                nc.vector.tensor_tensor(
                    ut2p[:, :, :, :], rtp[:, :, :, :],
                    psuS[:, 0:1, :, :].to_broadcast(bck), ALU.mult,
                )
                hinp = scp.tile(bck, BF16, tag="hinp")
                nc.vector.tensor_tensor(
                    hinp[:, :, :, :], ut2p[:, :, :, :],
                    pre_sb[:, 1, s0:s0 + BLK, :, :], ALU.add,
                )
                htp = scp.tile(bck, BF16, tag="htp")
                nc.scalar.activation(htp[:, :, :, :], hinp[:, :, :, :], AF.Tanh)
                return htp

            # rsum (+gsum) of each block, kept so the state update of block k
            # can be emitted later (it runs inside block k+1's sigmoid gap)
            rsums = {}
            gsums = {}

            def blk_tail(htp, k):
                """telescoped state delta of block k:
                D = sum_t w_t h_t - (sum_t w_t) C_{s0}"""
                s0 = BLK * k
                xh = scp.tile(bck, BF16, tag="xh")
                nc.vector.tensor_tensor(
                    xh[:, :, :, :], htp[:, :, :, :],
                    abc[:, s0:s0 + BLK, :, :].to_broadcast(bck), ALU.mult,
                )
                half = BLK // 2
                cur = xh
                width = BLK
                lvl = 0
                while width > 2:
                    nxt = scp.tile([128, width // 2, hc, b_loc], F32,
                                   tag=f"red{lvl}")
                    nc.vector.tensor_add(
                        nxt[:, :, :, :], cur[:, 0:width // 2, :, :],
                        cur[:, width // 2:width, :, :],
                    )
                    cur = nxt
                    width //= 2
                    lvl += 1
                if k == 0:
                    # C_0 = 0: D = rsum directly
                    gsum = scp.tile([128, hc, b_loc], BF16, tag="gsum")
                    nc.vector.tensor_add(
                        gsum[:, :, :], cur[:, 0, :, :], cur[:, 1, :, :]
                    )
                    gsums[k] = gsum
                else:
                    rsum = scp.tile([128, hc, b_loc], F32, tag="rsum")
                    nc.vector.tensor_add(
                        rsum[:, :, :], cur[:, 0, :, :], cur[:, 1, :, :]
                    )
                    gsum = scp.tile([128, hc, b_loc], BF16, tag="gsum")
                    nc.vector.tensor_sub(
                        gsum[:, :, :], rsum[:, :, :], wc[:, 0, :, :]
                    )
                    gsums[k] = gsum
                return gsums[k]

            def emit_state_update(k):
                """ct/wc update for block k (depends on gsum_k); emitted one
                block later so it runs inside the sigmoid gap."""
                nonlocal ct, wc
                if k < 0 or k not in gsums:
                    return
                gsum = gsums.pop(k)
                ct_new = scp.tile(bc1, F32, tag="ct")
                if ct is None:
                    nc.vector.tensor_copy(ct_new[:, 0, :, :], gsum[:, :, :])
                else:
                    nc.vector.tensor_add(
                        ct_new[:, 0, :, :], ct[:, 0, :, :], gsum[:, :, :]
                    )
                ct = ct_new
                if k + 1 < n_blk:
                    wc_new = scp.tile(bc1, F32, tag="wc")
                    nc.vector.tensor_tensor(
                        wc_new[:, 0, :, :], ct[:, 0, :, :],
                        wsc[:, k + 1:k + 2, 0, :].to_broadcast(
                            [128, hc, b_loc]
                        ),
                        ALU.mult,
                    )
                    wc = wc_new

            # ---- block 0 and 1: gates use C_0 = 0 (psr=0, psu=bu) ----
            htp = chain(0, None, bu_bc)
            gdp = blk_tail(htp, 0)

            psrS_prev = None
            psuS_prev = None
            for k in range(1, n_blk):
                last = k == n_blk - 1
                # PE: accumulate W^T @ D_{k-1} into psr then psu
                for gate in range(2):
                    ps = psr if gate == 0 else psu
                    for m in range(hc):
                        for c in range(hc):
                            nc.tensor.matmul(
                                ps[:, 0, m, :],
                                wb[:, c * 2 * h + gate * h + m * 128:
                                   c * 2 * h + gate * h + (m + 1) * 128],
                                gdp[:, c, :],
                                start=(gate == 0 and k == 1 and c == 0),
                                stop=(last and c == hc - 1),
                                skip_group_check=True,
                            )
                if k == 1:
                    # block 1 gates also use C_0 (snapshots not ready yet)
                    htp = chain(1, None, bu_bc)
                else:
                    htp = chain(k, psrS_prev, psuS_prev)
                # snapshot psr/psu after this block's matmuls, for block k+1's
                # gates (runs inside the tanh gap on the DVE)
                if not last:
                    psrS = scp.tile(bc1, BF16, tag="psrS")
                    nc.vector.tensor_copy(psrS[:, :, :, :], psr[:, :, :, :])
                    psuS = scp.tile(bc1, BF16, tag="psuS")
                    nc.vector.tensor_copy(psuS[:, :, :, :], psu[:, :, :, :])
                    psrS_prev, psuS_prev = psrS, psuS
                gdp = blk_tail(htp, k)

            # final C = C_{S} (flush the last two state updates)
            emit_state_update(n_blk - 2)
            emit_state_update(n_blk - 1)
            cfin = scp.tile([128, hc, b_loc], F32R, tag="cfin")
            nc.vector.tensor_copy(cfin[:, :, :], ct[:, 0, :, :])

            # ============ P6: next memory ============
            po = ops.tile([b_loc, h], F32, tag="po")
            # mT/qT are b-major [128, b, c]; cfin is [128, c, b]
            chunks = [(mT, True), (cfin, False), (qT, True)]
            for part, (src, bmajor) in enumerate(chunks):
                for c in range(hc):
                    j = part * hc + c
                    nc.tensor.matmul(
                        po[:, :],
                        src[:, :, c] if bmajor else src[:, c, :],
                        nmw[:, j * h:(j + 1) * h],
                        start=(j == 0), stop=False,
                    )
            nc.tensor.matmul(
                po[:, :], ones_row[:, :], nmb_row[:, :], start=False, stop=True
            )
            out_sb = scp.tile([b_loc, h], F32, tag="out_sb")
            nc.scalar.activation(out_sb[:, :], po[:, :], AF.Relu)
            nc.sync.dma_start(io["out"][:, 0, :], out_sb[:, :])


_NC_CACHE = {}


def _run(inputs, **spmd_kwargs):
    if "full" not in _NC_CACHE:
        _NC_CACHE["full"] = build_nc()
    nc = _NC_CACHE["full"]

    names = ["facts", "questions", "prevM", "z1_w", "z1_b", "z2_w",
             "Wr", "br", "Ur", "bur", "W", "bw", "U", "bu", "nm_w", "nm_b"]
    sharded = {"facts", "questions", "prevM"}
    in_maps = []
    for i in range(N_CORES):
        m = {}
        for n in names:
            v = np.asarray(inputs[n], dtype=np.float32)
            if n in sharded:
                v = v[i * B_LOC:(i + 1) * B_LOC]
            m[n] = np.ascontiguousarray(v)
        in_maps.append(m)

    res = run_bass_kernel_spmd(nc, in_maps, list(range(N_CORES)), **spmd_kwargs)
    out = np.concatenate(
        [res.results[i]["out"] for i in range(N_CORES)], axis=0
    ).astype(np.float32)
    return out, res


def kernel(**inputs):
    return _run(inputs)[0]


# revision 44
# speedup vs baseline: 1.1353x; 1.1353x over previous
"""Trainium2 Bass kernel for EpisodicMemory (DMN episodic memory module).

Full shapes: facts (128,256,512), questions/prevM (128,1,512), output (128,1,512).
Sharding: data-parallel over batch, 16 batches per core x 8 cores, weights
replicated. Everything on-chip (no DRAM scratch): activations are kept
feature-on-partition ("transposed") so matmuls contract over the partition
dim and pointwise ops run 128 lanes wide.

Per-core pipeline:
  P1  transpose facts to fT via PE transposes (2-batch groups)
  P2  interaction features zT (bf16) -> z1 MLP (tanh) -> z2 logits
  P3  pre_r = Wr@f + (br+bur), pre_h = W@f + bw   (stored bf16 in SBUF)
  P4  softmax over S -> G; broadcast G to all partitions (DRAM bounce)
  P5  GRU scan over S=256 steps in delta form: psr/psu live in PSUM for the
      whole scan and accumulate W^T @ gd_s (gd = per-step state delta, bf16).
      Weights bf16 (1 cycle/row vs 4 for f32r). bu folded into the psu PSUM
      init via K=1 matmuls. psr-gate matmuls issue first so the sigmoid path
      overlaps the psu-gate matmuls.
  P6  next_mem = relu([prevM C q] @ nm_w + nm_b) via C-stationary matmuls
"""

from contextlib import ExitStack

import numpy as np

import concourse.bass as bass
import concourse.tile as tile
from concourse import bacc, masks, mybir
from concourse.bass_utils import run_bass_kernel_spmd

F32 = mybir.dt.float32
F32R = mybir.dt.float32r
BF16 = mybir.dt.bfloat16
FP8 = mybir.dt.float8e4
AF = mybir.ActivationFunctionType
ALU = mybir.AluOpType
DR = mybir.MatmulPerfMode.DoubleRow
Z1SC = 16.0  # fp8 scale for z1_w (values ~N(0, 0.02) -> normal e4m3 range)

B, S, H = 128, 256, 512
N_CORES = 8
B_LOC = B // N_CORES  # 16


def build_nc(b_loc=B_LOC, s_len=S):
    """Build the per-core Bass program (SPMD: same program, sharded data)."""
    h = H
    nc = bacc.Bacc(
        "TRN2", target_bir_lowering=False, debug=False, num_devices=N_CORES
    )

    io = {}
    io["facts"] = nc.dram_tensor("facts", [b_loc, s_len, h], F32, kind="ExternalInput")
    io["questions"] = nc.dram_tensor("questions", [b_loc, 1, h], F32, kind="ExternalInput")
    io["prevM"] = nc.dram_tensor("prevM", [b_loc, 1, h], F32, kind="ExternalInput")
    io["z1_w"] = nc.dram_tensor("z1_w", [4 * h, h], F32, kind="ExternalInput")
    io["z1_b"] = nc.dram_tensor("z1_b", [h], F32, kind="ExternalInput")
    io["z2_w"] = nc.dram_tensor("z2_w", [h, 1], F32, kind="ExternalInput")
    for nm in ["Wr", "Ur", "W", "U"]:
        io[nm] = nc.dram_tensor(nm, [h, h], F32, kind="ExternalInput")
    for nm in ["br", "bur", "bw", "bu"]:
        io[nm] = nc.dram_tensor(nm, [h], F32, kind="ExternalInput")
    io["nm_w"] = nc.dram_tensor("nm_w", [3 * h, h], F32, kind="ExternalInput")
    io["nm_b"] = nc.dram_tensor("nm_b", [h], F32, kind="ExternalInput")
    io["out"] = nc.dram_tensor("out", [b_loc, 1, h], F32, kind="ExternalOutput")
    io["g_bounce"] = nc.dram_tensor("g_bounce", [s_len + s_len // 8, b_loc], F32)
    io["logit_dram"] = nc.dram_tensor("logit_dram", [b_loc, s_len], F32)

    with tile.TileContext(nc) as tc:
        _body(tc, io, b_loc, s_len, h)
    nc.compile()
    return nc


def _body(tc, io, b_loc, s_len, h):
    nc = tc.nc
    hc = h // 128          # 4 h-chunks
    zc = 4 * hc            # 16 chunks of the 4H interaction dim
    gb = 2                 # batches per group (matmul moving dim = gb*s_len)
    ng = b_loc // gb
    sc_ = s_len // 128

    facts, questions, prevM = io["facts"], io["questions"], io["prevM"]

    with ExitStack() as ctx:
        # ---------------- resident pools ----------------
        wpool = ctx.enter_context(tc.tile_pool(name="wres", bufs=1))
        prepool = ctx.enter_context(tc.tile_pool(name="prepool", bufs=1))
        smallpool = ctx.enter_context(tc.tile_pool(name="small", bufs=1))
        pfpool = ctx.enter_context(tc.tile_pool(name="pf", bufs=1))

        # prefetch the first 2-batch group of facts ahead of the weight DMAs
        # so the PE can start transposing ~40us earlier
        fpre = pfpool.tile([128, gb * sc_, h], F32, tag="fpre")
        for bp in range(gb):
            for sh in range(sc_):
                nc.sync.dma_start(
                    fpre[:, bp * sc_ + sh, :],
                    facts[bp, sh * 128:(sh + 1) * 128, :],
                )

        # scan gate weights [Ur | U]: k-chunk c at cols [c*2h, (c+1)*2h)
        # (DMAs are emitted in P5 so they don't delay the P1/P2 startup)
        wcomb = wpool.tile([128, hc * 2 * h], F32R, tag="wcomb")

        # small constants: (128, hc) with col = h-chunk
        def load_cvec(nm):
            t = smallpool.tile([128, hc], F32, tag=f"cv_{nm}")
            nc.sync.dma_start(t[:, :], io[nm].rearrange("(c p) -> p c", p=128))
            return t

        z1b4 = load_cvec("z1_b")
        br4 = load_cvec("br")
        bur4 = load_cvec("bur")
        bw4 = load_cvec("bw")
        z2c = smallpool.tile([128, hc], BF16, tag="z2c")
        z2stg = smallpool.tile([128, hc], F32, tag="z2stg")
        nc.sync.dma_start(
            z2stg[:, :], io["z2_w"].rearrange("(c p) o -> p (c o)", p=128)
        )
        nc.vector.tensor_copy(z2c[:, :], z2stg[:, :])
        brc4 = smallpool.tile([128, hc], F32, tag="brc4")  # br + bur
        nc.vector.tensor_copy(brc4[:, :], br4[:, :])
        nc.vector.tensor_add(brc4[:, :], brc4[:, :], bur4[:, :])

        # bu as a row [1, h] (bf16) + ones row for psu PSUM bias init
        bu_stg = smallpool.tile([1, h], F32, tag="bu_stg")
        nc.sync.dma_start(bu_stg[:, :], io["bu"][None, :])
        bu_row = smallpool.tile([1, h], BF16, tag="bu_row")
        nc.vector.tensor_copy(bu_row[:, :], bu_stg[:, :])
        onesb_stg = smallpool.tile([1, b_loc], F32, tag="onesb_stg")
        nc.vector.memset(onesb_stg[:, :], 1.0)
        onesb = smallpool.tile([1, b_loc], BF16, tag="onesb")
        nc.vector.tensor_copy(onesb[:, :], onesb_stg[:, :])

        # questions / prevM transposed, b-major free layout (128, b_loc, hc)
        # so the gather merges into one DMA descriptor each; the dma_start
        # calls are emitted after the z1 weight load (they'd block the sync
        # queue ~6us each otherwise)
        qT = smallpool.tile([128, b_loc, hc], F32R, tag="qT")
        mT = smallpool.tile([128, b_loc, hc], F32R, tag="mT")
        nqT = smallpool.tile([128, b_loc, hc], F32, tag="nqT")
        nmT = smallpool.tile([128, b_loc, hc], F32, tag="nmT")

        ones_row = smallpool.tile([1, b_loc], F32R, tag="ones_row")
        nc.vector.tensor_copy(ones_row[:, :], onesb_stg[:, :])
        nmb_row = smallpool.tile([1, h], F32R, tag="nmb_row")
        nc.sync.dma_start(nmb_row[:, :], io["nm_b"][None, :].bitcast(F32R))

        ident = smallpool.tile([128, 128], F32, tag="ident")
        masks.make_identity(nc, ident[:, :])

        # pre-activations resident through the scan: [p, gate, s, m, b] bf16
        # (s-major so the per-step slice [m, b] is contiguous)
        pre_sb = prepool.tile([128, 2, s_len, hc, b_loc], BF16, tag="pre_sb")
        logit = smallpool.tile([b_loc, s_len], F32, tag="logit")

        # ============ phases P1..P3 (per 2-batch group) ============
        with (
            tc.tile_pool(name="phw", bufs=1) as phw,
            tc.tile_pool(name="ph", bufs=2) as ph,
            tc.tile_pool(name="zpool", bufs=3) as zp,
            tc.tile_pool(name="ghpool", bufs=1) as ghpool,
            tc.tile_pool(name="tps", bufs=2, space="PSUM") as tps,
            tc.tile_pool(name="ghps", bufs=1, space="PSUM") as ghps,
            tc.tile_pool(name="lgps", bufs=1, space="PSUM") as lgps,
        ):
            # z1 weights in fp8e4 (scaled by Z1SC; staged through f32),
            # laid out [128, k-tile, h] for DoubleRow matmuls. One big DMA +
            # one big cast — a chunked DMA/cast pipeline here stalls the
            # whole P2 startup on staging-buffer reuse.
            # z1 weights: 2 chunked DMA+cast stages (the first DoubleRow
            # matmuls need only the early k-tiles); q/m gathers (slow
            # 4B-element DMAs) go after the weights
            z1stg = phw.tile([128, zc, h], F32, tag="z1stg")
            z1w = phw.tile([128, zc, h], FP8, tag="z1w")
            zw_src = io["z1_w"].rearrange("(k p) h -> p k h", p=128)
            for ch in range(2):
                kk = slice(8 * ch, 8 * ch + 8)
                nc.sync.dma_start(z1stg[:, kk, :], zw_src[:, kk, :])
                nc.vector.tensor_scalar_mul(
                    z1w[:, kk, :], z1stg[:, kk, :], Z1SC
                )
            nc.sync.dma_start(
                qT[:, :, :],
                questions[:, 0, :].rearrange(
                    "b (c p) -> p b c", p=128).bitcast(F32R),
            )
            nc.sync.dma_start(
                mT[:, :, :],
                prevM[:, 0, :].rearrange(
                    "b (c p) -> p b c", p=128).bitcast(F32R),
            )
            nc.vector.tensor_scalar_mul(
                nqT[:, :, :], qT[:, :, :].bitcast(F32), -1.0
            )
            nc.vector.tensor_scalar_mul(
                nmT[:, :, :], mT[:, :, :].bitcast(F32), -1.0
            )
            # row-sums of Wr / W (the reference's einsum "bsh,hk->bsh"
            # multiplies facts elementwise by these row-sums)
            rs4 = phw.tile([128, hc, 2], F32, tag="rs4")
            for gate, wname in enumerate(["Wr", "W"]):
                wstg = phw.tile([128, hc, h], F32, tag=f"wstg{gate}")
                nc.sync.dma_start(
                    wstg[:, :, :],
                    io[wname].rearrange("(c p) h -> p c h", p=128),
                )
                for c in range(hc):
                    nc.vector.tensor_reduce(
                        rs4[:, c, gate:gate + 1], wstg[:, c, :],
                        mybir.AxisListType.X, ALU.add,
                    )

            for g in range(ng):
                # ---- P1: load + transpose facts ----
                fT2 = ph.tile([128, hc, gb * s_len], BF16, tag="fT2")
                for bp in range(gb):
                    bi = g * gb + bp
                    for sh in range(sc_):
                        if g == 0:
                            fnat = fpre[:, bp * sc_ + sh, :]
                        else:
                            fnat = ph.tile([128, h], F32, tag="fnat")
                            nc.sync.dma_start(
                                fnat[:, :], facts[bi, sh * 128:(sh + 1) * 128, :]
                            )
                        tp = tps.tile([128, hc, 128], F32, tag="tpsum")
                        for c in range(hc):
                            nc.tensor.transpose(
                                tp[:, c, :], fnat[:, c * 128:(c + 1) * 128],
                                ident[:, :],
                            )
                        nc.vector.tensor_copy(
                            fT2[:, :, bp * s_len + sh * 128:
                                bp * s_len + (sh + 1) * 128],
                            tp[:, :, :],
                        )

                # ---- P2: interaction features (fp8) + z1 MLP (DoubleRow)
                #      + z2 logits ----
                ghp = [ghps.tile([128, gb * s_len], F32, name=f"ghp{m}",
                                 tag=f"ghp{m}") for m in range(hc)]
                for j in range(zc // 2):
                    zk2 = zp.tile([128, 2, gb * s_len], FP8, tag="zk")
                    for half in range(2):
                        k = 2 * j + half
                        kind, c = divmod(k, hc)  # 0:f*q 1:f*m 2:|f-q| 3:|f-m|
                        for bp in range(gb):
                            bi = g * gb + bp
                            dst = zk2[:, half, bp * s_len:(bp + 1) * s_len]
                            src = fT2[:, c, bp * s_len:(bp + 1) * s_len]
                            if kind == 0:
                                nc.vector.tensor_scalar_mul(
                                    dst, src, qT[:, bi, c:c + 1].bitcast(F32))
                            elif kind == 1:
                                nc.vector.tensor_scalar_mul(
                                    dst, src, mT[:, bi, c:c + 1].bitcast(F32))
                            elif kind == 2:
                                nc.scalar.activation(dst, src, AF.Abs,
                                                     bias=nqT[:, bi, c:c + 1])
                            else:
                                nc.scalar.activation(dst, src, AF.Abs,
                                                     bias=nmT[:, bi, c:c + 1])
                    for m in range(hc):
                        nc.tensor.matmul(
                            ghp[m][:, :],
                            z1w[:, 2 * j:2 * j + 2, m * 128:(m + 1) * 128],
                            zk2[:, :, :],
                            start=(j == 0),
                            stop=(j == zc // 2 - 1),
                            perf_mode=DR,
                        )
                ghT = ph.tile([128, hc, gb * s_len], BF16, tag="ghT")
                for m in range(hc):
                    nc.scalar.activation(
                        ghT[:, m, :], ghp[m][:, :], AF.Tanh,
                        bias=z1b4[:, m:m + 1], scale=1.0 / Z1SC,
                    )
                lgp = lgps.tile([1, gb * s_len], F32, tag="lgp")
                for m in range(hc):
                    nc.tensor.matmul(
                        lgp[:, :], z2c[:, m:m + 1], ghT[:, m, :],
                        start=(m == 0), stop=(m == hc - 1),
                    )
                lstage = ph.tile([1, gb * s_len], F32, tag="lstage")
                nc.vector.tensor_copy(lstage[:, :], lgp[:, :])
                nc.sync.dma_start(
                    io["logit_dram"][None, g * gb:(g + 1) * gb, :],
                    lstage[:, :].rearrange("o (b s) -> o b s", b=gb),
                )

                # ---- P3: pre_r / pre_h = facts * rowsum(W) + bias ----
                for gate in range(2):
                    bias4 = brc4 if gate == 0 else bw4
                    for m in range(hc):
                        nc.vector.tensor_scalar(
                            pre_sb[:, gate, :, m, g * gb:(g + 1) * gb],
                            fT2[:, m, :].rearrange("p (b s) -> p s b", b=gb),
                            rs4[:, m, gate:gate + 1],
                            bias4[:, m:m + 1],
                            ALU.mult,
                            ALU.add,
                        )

        # ============ P4: softmax over S + blend-weight broadcast ============
        # The scan processes blocks of BLK steps with C frozen per block; the
        # within-block state update telescopes exactly:
        #   D_BLK = sum_t w_t * e_t,  w_t = g_t * prod_{u>t} (1 - g_u)
        # Compute w in (b, s) layout, then broadcast to all partitions.
        BLK = 8
        abw = smallpool.tile([128, s_len + s_len // BLK, 1, b_loc], F32, tag="abw")
        abc = abw[:, :s_len, :, :]     # w_t broadcast
        wsc = abw[:, s_len:, :, :]     # per-block sum_t w_t broadcast
        with tc.tile_pool(name="smax", bufs=1) as sp:
            nc.sync.dma_start(logit[:, :], io["logit_dram"][:, :])
            negmax = sp.tile([b_loc, 1], F32, tag="negmax")
            nc.vector.tensor_reduce(
                negmax[:, :], logit[:, :], mybir.AxisListType.X, ALU.max, negate=True
            )
            esum = sp.tile([b_loc, 1], F32, tag="esum")
            gexp = sp.tile([b_loc, s_len], F32, tag="gexp")
            nc.scalar.activation(
                gexp[:, :], logit[:, :], AF.Exp, bias=negmax[:, :],
                accum_out=esum[:, :],
            )
            inv = sp.tile([b_loc, 1], F32, tag="inv")
            nc.vector.reciprocal(inv[:, :], esum[:, :])
            gmat = sp.tile([b_loc, s_len], F32, tag="gmat")
            nc.vector.tensor_scalar_mul(gmat[:, :], gexp[:, :], inv[:, :])

            # blend weights w_t = g_t * prod_{u>t in blk} (1-g_u) via suffix
            # products within each BLK-sized block
            om = sp.tile([b_loc, s_len], F32, tag="om")
            nc.vector.tensor_scalar(
                om[:, :], gmat[:, :], -1.0, 1.0, ALU.mult, ALU.add
            )
            wmat = sp.tile([b_loc, s_len], F32, tag="wmat")
            nbk = s_len // BLK
            nc.vector.tensor_copy(wmat[:, BLK - 1::BLK], gmat[:, BLK - 1::BLK])
            suf = sp.tile([b_loc, nbk], F32, tag="suf_a")
            nc.vector.tensor_copy(suf[:, :], om[:, BLK - 1::BLK])
            for t in range(BLK - 2, -1, -1):
                nc.vector.tensor_tensor(
                    wmat[:, t::BLK], gmat[:, t::BLK], suf[:, :], ALU.mult
                )
                if t > 0:
                    suf2 = sp.tile(
                        [b_loc, nbk], F32,
                        tag="suf_b" if (BLK - t) % 2 == 0 else "suf_a",
                    )
                    nc.vector.tensor_tensor(
                        suf2[:, :], om[:, t::BLK], suf[:, :], ALU.mult
                    )
                    suf = suf2
            # per-block sum of the w_t (pairwise reduce)
            cur = wmat
            width = s_len
            lvl = 0
            while width > nbk:
                nxt = sp.tile([b_loc, width // 2], F32, tag=f"ws{lvl}")
                nc.vector.tensor_tensor(
                    nxt[:, :], cur[:, 0::2], cur[:, 1::2], ALU.add
                )
                cur = nxt
                width //= 2
                lvl += 1

            # broadcast w + wsum to all partitions through a DRAM bounce
            nc.sync.dma_start(
                io["g_bounce"][:s_len, :].rearrange("s b -> b s"), wmat[:, :]
            )
            nc.sync.dma_start(
                io["g_bounce"][s_len:, :].rearrange("s b -> b s"), cur[:, :]
            )
            nc.sync.dma_start(
                abw[:, :, 0, :],
                io["g_bounce"][None, :, :].to_broadcast(
                    [128, s_len + s_len // BLK, b_loc]
                ),
            )

        # ============ P5: GRU scan (blocked steps, delta-PSUM form) ============
        # Steps are processed in blocks of BLK. The gate pre-activations for
        # block k use the state snapshot taken after block k-1's matmuls,
        # which itself lags one block (two-block-stale gates, <= 2*BLK-1
        # steps; g ~ 1/256 keeps the error ~1e-3 relative). This decouples
        # the PE pass and the PSUM->SBUF snapshot from the serial DVE/ACT
        # chain: they hide inside the sigmoid/tanh gaps. The within-block
        # state update is EXACT via the telescoped form
        #   D = sum_t w_t h_t - (sum_t w_t) C_s0.
        # psr/psu live in PSUM the whole scan and accumulate W^T @ D once per
        # block.
        n_blk = s_len // BLK
        with (
            tc.tile_pool(name="scw", bufs=1) as scw,
            tc.tile_pool(name="scan_sb", bufs=2) as scp,
            tc.tile_pool(name="scan_ps", bufs=1, space="PSUM") as sps,
            tc.tile_pool(name="out_ps", bufs=1, space="PSUM") as ops,
        ):
            # load + cast the scan weights to bf16 (1 cycle/row on the PE vs
            # 4 for f32r at narrow moving); DMAs deferred to here so the
            # P1/P2 startup isn't delayed
            for c in range(hc):
                nc.sync.dma_start(
                    wcomb[:, c * 2 * h: c * 2 * h + h],
                    io["Ur"][c * 128:(c + 1) * 128, :].bitcast(F32R),
                )
                nc.sync.dma_start(
                    wcomb[:, c * 2 * h + h: (c + 1) * 2 * h],
                    io["U"][c * 128:(c + 1) * 128, :].bitcast(F32R),
                )
            wb = scw.tile([128, hc * 2 * h], BF16, tag="wb")
            nc.vector.tensor_copy(wb[:, :], wcomb[:, :].bitcast(F32))

            # final-layer weights (loaded while the scan runs)
            nmw = scw.tile([128, 3 * hc * h], F32R, tag="nmw")
            for j in range(3 * hc):
                nc.sync.dma_start(
                    nmw[:, j * h:(j + 1) * h],
                    io["nm_w"][j * 128:(j + 1) * 128, :].bitcast(F32R),
                )

            # persistent PSUM accumulators: psr/psu = W^T @ C_{2k} (+ bu)
            psr = sps.tile([128, 1, hc, b_loc], F32, tag="psr")
            psu = sps.tile([128, 1, hc, b_loc], F32, tag="psu")
            # psu init = bu (broadcast over batch) via K=1 matmuls; psr gets
            # start=True on its first real matmul (pair k=1)
            for m in range(hc):
                nc.tensor.matmul(
                    psu[:, 0, m, :], bu_row[:, m * 128:(m + 1) * 128],
                    onesb[:, :], start=True, stop=False, skip_group_check=True,
                )

            ct = None     # C_{s0} (f32), set by the first tail
            wc = None     # (sum w)*C snapshot for the gsum of the next block
            bck = [128, BLK, hc, b_loc]
            bc1 = [128, 1, hc, b_loc]

            # bu broadcast for the first two blocks' gate math
            bu4 = load_cvec("bu")
            bu_bc = scw.tile(bc1, F32, tag="bu_bc")
            ones_pb = scw.tile([128, b_loc], F32, tag="ones_pb")
            nc.vector.memset(ones_pb[:, :], 1.0)
            for c in range(hc):
                nc.vector.tensor_scalar_mul(
                    bu_bc[:, 0, c, :], ones_pb[:, :], bu4[:, c:c + 1]
                )

            def chain(k, psrS, psuS):
                """gate math for block k using (stale) state snapshots;
                emits the ct/wc updates of block k-1 into the sigmoid gap."""
                s0 = BLK * k
                if psrS is None:
                    rtp = scp.tile(bck, BF16, tag="rtp")
                    nc.scalar.activation(
                        rtp[:, :, :, :], pre_sb[:, 0, s0:s0 + BLK, :, :],
                        AF.Sigmoid,
                    )
                else:
                    sgp = scp.tile(bck, BF16, tag="sgp")
                    nc.vector.tensor_tensor(
                        sgp[:, :, :, :],
                        psrS[:, 0:1, :, :].to_broadcast(bck),
                        pre_sb[:, 0, s0:s0 + BLK, :, :], ALU.add,
                    )
                    rtp = scp.tile(bck, BF16, tag="rtp")
                    nc.scalar.activation(
                        rtp[:, :, :, :], sgp[:, :, :, :], AF.Sigmoid
                    )
                # ct/wc updates of the previous block run during the sigmoid
                emit_state_update(k - 1)
                ut2p = scp.tile(bck, BF16, tag="ut2p")
                nc.vector.tensor_tensor(
                    ut2p[:, :, :, :], rtp[:, :, :, :],
                    psuS[:, 0:1, :, :].to_broadcast(bck), ALU.mult,
                )
                hinp = scp.tile(bck, BF16, tag="hinp")
                nc.vector.tensor_tensor(
                    hinp[:, :, :, :], ut2p[:, :, :, :],
                    pre_sb[:, 1, s0:s0 + BLK, :, :], ALU.add,
                )
                htp = scp.tile(bck, BF16, tag="htp")
                nc.scalar.activation(htp[:, :, :, :], hinp[:, :, :, :], AF.Tanh)
                return htp

            # rsum (+gsum) of each block, kept so the state update of block k
            # can be emitted later (it runs inside block k+1's sigmoid gap)
            rsums = {}
            gsums = {}

            def blk_tail(htp, k):
                """telescoped state delta of block k:
                D = sum_t w_t h_t - (sum_t w_t) C_{s0}"""
                s0 = BLK * k
                xh = scp.tile(bck, BF16, tag="xh")
                nc.vector.tensor_tensor(
                    xh[:, :, :, :], htp[:, :, :, :],
                    abc[:, s0:s0 + BLK, :, :].to_broadcast(bck), ALU.mult,
                )
                half = BLK // 2
                cur = xh
                width = BLK
                lvl = 0
                while width > 2:
                    nxt = scp.tile([128, width // 2, hc, b_loc], F32,
                                   tag=f"red{lvl}")
                    nc.vector.tensor_add(
                        nxt[:, :, :, :], cur[:, 0:width // 2, :, :],
                        cur[:, width // 2:width, :, :],
                    )
                    cur = nxt
                    width //= 2
                    lvl += 1
                if k == 0:
                    # C_0 = 0: D = rsum directly
                    gsum = scp.tile([128, hc, b_loc], BF16, tag="gsum")
                    nc.vector.tensor_add(
                        gsum[:, :, :], cur[:, 0, :, :], cur[:, 1, :, :]
                    )
                    gsums[k] = gsum
                else:
                    rsum = scp.tile([128, hc, b_loc], F32, tag="rsum")
                    nc.vector.tensor_add(
                        rsum[:, :, :], cur[:, 0, :, :], cur[:, 1, :, :]
                    )
                    gsum = scp.tile([128, hc, b_loc], BF16, tag="gsum")
                    nc.vector.tensor_sub(
                        gsum[:, :, :], rsum[:, :, :], wc[:, 0, :, :]
                    )
                    gsums[k] = gsum
                return gsums[k]

            def emit_state_update(k):
                """ct/wc update for block k (depends on gsum_k); emitted one
                block later so it runs inside the sigmoid gap."""
                nonlocal ct, wc
                if k < 0 or k not in gsums:
                    return
                gsum = gsums.pop(k)
                ct_new = scp.tile(bc1, F32, tag="ct")
                if ct is None:
                    nc.vector.tensor_copy(ct_new[:, 0, :, :], gsum[:, :, :])
                else:
                    nc.vector.tensor_add(
                        ct_new[:, 0, :, :], ct[:, 0, :, :], gsum[:, :, :]
                    )
                ct = ct_new
                if k + 1 < n_blk:
                    wc_new = scp.tile(bc1, F32, tag="wc")
                    nc.vector.tensor_tensor(
                        wc_new[:, 0, :, :], ct[:, 0, :, :],
                        wsc[:, k + 1:k + 2, 0, :].to_broadcast(
                            [128, hc, b_loc]
                        ),
                        ALU.mult,
                    )
                    wc = wc_new

            # ---- block 0 and 1: gates use C_0 = 0 (psr=0, psu=bu) ----
            htp = chain(0, None, bu_bc)
            gdp = blk_tail(htp, 0)

            psrS_prev = None
            psuS_prev = None
            for k in range(1, n_blk):
                last = k == n_blk - 1
                # PE: accumulate W^T @ D_{k-1} into psr then psu
                for gate in range(2):
                    ps = psr if gate == 0 else psu
                    for m in range(hc):
                        for c in range(hc):
                            nc.tensor.matmul(
                                ps[:, 0, m, :],
                                wb[:, c * 2 * h + gate * h + m * 128:
                                   c * 2 * h + gate * h + (m + 1) * 128],
                                gdp[:, c, :],
                                start=(gate == 0 and k == 1 and c == 0),
                                stop=(last and c == hc - 1),
                                skip_group_check=True,
                            )
                if k == 1:
                    # block 1 gates also use C_0 (snapshots not ready yet)
                    htp = chain(1, None, bu_bc)
                else:
                    htp = chain(k, psrS_prev, psuS_prev)
                # snapshot psr/psu after this block's matmuls, for block k+1's
                # gates (runs inside the tanh gap on the DVE)
                if not last:
                    psrS = scp.tile(bc1, BF16, tag="psrS")
                    nc.vector.tensor_copy(psrS[:, :, :, :], psr[:, :, :, :])
                    psuS = scp.tile(bc1, BF16, tag="psuS")
                    nc.vector.tensor_copy(psuS[:, :, :, :], psu[:, :, :, :])
                    psrS_prev, psuS_prev = psrS, psuS
                gdp = blk_tail(htp, k)

            # final C = C_{S} (flush the last two state updates)
            emit_state_update(n_blk - 2)
            emit_state_update(n_blk - 1)
            cfin = scp.tile([128, hc, b_loc], F32R, tag="cfin")
            nc.vector.tensor_copy(cfin[:, :, :], ct[:, 0, :, :])

            # ============ P6: next memory ============
            po = ops.tile([b_loc, h], F32, tag="po")
            # mT/qT are b-major [128, b, c]; cfin is [128, c, b]
            chunks = [(mT, True), (cfin, False), (qT, True)]
            for part, (src, bmajor) in enumerate(chunks):
                for c in range(hc):
                    j = part * hc + c
                    nc.tensor.matmul(
                        po[:, :],
                        src[:, :, c] if bmajor else src[:, c, :],
                        nmw[:, j * h:(j + 1) * h],
                        start=(j == 0), stop=False,
                    )
            nc.tensor.matmul(
                po[:, :], ones_row[:, :], nmb_row[:, :], start=False, stop=True
            )
            out_sb = scp.tile([b_loc, h], F32, tag="out_sb")
            nc.scalar.activation(out_sb[:, :], po[:, :], AF.Relu)
            nc.sync.dma_start(io["out"][:, 0, :], out_sb[:, :])


_NC_CACHE = {}


def _run(inputs, **spmd_kwargs):
    if "full" not in _NC_CACHE:
        _NC_CACHE["full"] = build_nc()
    nc = _NC_CACHE["full"]

    names = ["facts", "questions", "prevM", "z1_w", "z1_b", "z2_w",
             "Wr", "br", "Ur", "bur", "W", "bw", "U", "bu", "nm_w", "nm_b"]
    sharded = {"facts", "questions", "prevM"}
    in_maps = []
    for i in range(N_CORES):
        m = {}
        for n in names:
            v = np.asarray(inputs[n], dtype=np.float32)
            if n in sharded:
                v = v[i * B_LOC:(i + 1) * B_LOC]
            m[n] = np.ascontiguousarray(v)
        in_maps.append(m)

    res = run_bass_kernel_spmd(nc, in_maps, list(range(N_CORES)), **spmd_kwargs)
    out = np.concatenate(
        [res.results[i]["out"] for i in range(N_CORES)], axis=0
    ).astype(np.float32)
    return out, res


def kernel(**inputs):
    return _run(inputs)[0]


# revision 45
# speedup vs baseline: 1.1566x; 1.0187x over previous
"""Trainium2 Bass kernel for EpisodicMemory (DMN episodic memory module).

Full shapes: facts (128,256,512), questions/prevM (128,1,512), output (128,1,512).
Sharding: data-parallel over batch, 16 batches per core x 8 cores, weights
replicated. Everything on-chip (no DRAM scratch): activations are kept
feature-on-partition ("transposed") so matmuls contract over the partition
dim and pointwise ops run 128 lanes wide.

Per-core pipeline:
  P1  transpose facts to fT via PE transposes (2-batch groups)
  P2  interaction features zT (bf16) -> z1 MLP (tanh) -> z2 logits
  P3  pre_r = Wr@f + (br+bur), pre_h = W@f + bw   (stored bf16 in SBUF)
  P4  softmax over S -> G; broadcast G to all partitions (DRAM bounce)
  P5  GRU scan over S=256 steps in delta form: psr/psu live in PSUM for the
      whole scan and accumulate W^T @ gd_s (gd = per-step state delta, bf16).
      Weights bf16 (1 cycle/row vs 4 for f32r). bu folded into the psu PSUM
      init via K=1 matmuls. psr-gate matmuls issue first so the sigmoid path
      overlaps the psu-gate matmuls.
  P6  next_mem = relu([prevM C q] @ nm_w + nm_b) via C-stationary matmuls
"""

from contextlib import ExitStack

import numpy as np

import concourse.bass as bass
import concourse.tile as tile
from concourse import bacc, masks, mybir
from concourse.bass_utils import run_bass_kernel_spmd

F32 = mybir.dt.float32
F32R = mybir.dt.float32r
BF16 = mybir.dt.bfloat16
FP8 = mybir.dt.float8e4
AF = mybir.ActivationFunctionType
ALU = mybir.AluOpType
DR = mybir.MatmulPerfMode.DoubleRow
Z1SC = 16.0  # fp8 scale for z1_w (values ~N(0, 0.02) -> normal e4m3 range)

B, S, H = 128, 256, 512
N_CORES = 8
B_LOC = B // N_CORES  # 16


def build_nc(b_loc=B_LOC, s_len=S):
    """Build the per-core Bass program (SPMD: same program, sharded data)."""
    h = H
    nc = bacc.Bacc(
        "TRN2", target_bir_lowering=False, debug=False, num_devices=N_CORES
    )

    io = {}
    io["facts"] = nc.dram_tensor("facts", [b_loc, s_len, h], F32, kind="ExternalInput")
    io["questions"] = nc.dram_tensor("questions", [b_loc, 1, h], F32, kind="ExternalInput")
    io["prevM"] = nc.dram_tensor("prevM", [b_loc, 1, h], F32, kind="ExternalInput")
    io["z1_w"] = nc.dram_tensor("z1_w", [4 * h, h], F32, kind="ExternalInput")
    io["z1_b"] = nc.dram_tensor("z1_b", [h], F32, kind="ExternalInput")
    io["z2_w"] = nc.dram_tensor("z2_w", [h, 1], F32, kind="ExternalInput")
    for nm in ["Wr", "Ur", "W", "U"]:
        io[nm] = nc.dram_tensor(nm, [h, h], F32, kind="ExternalInput")
    for nm in ["br", "bur", "bw", "bu"]:
        io[nm] = nc.dram_tensor(nm, [h], F32, kind="ExternalInput")
    io["nm_w"] = nc.dram_tensor("nm_w", [3 * h, h], F32, kind="ExternalInput")
    io["nm_b"] = nc.dram_tensor("nm_b", [h], F32, kind="ExternalInput")
    io["out"] = nc.dram_tensor("out", [b_loc, 1, h], F32, kind="ExternalOutput")
    io["g_bounce"] = nc.dram_tensor("g_bounce", [s_len + s_len // 8, b_loc], F32)
    io["logit_dram"] = nc.dram_tensor("logit_dram", [b_loc, s_len], F32)

    with tile.TileContext(nc) as tc:
        _body(tc, io, b_loc, s_len, h)
    nc.compile()
    return nc


def _body(tc, io, b_loc, s_len, h):
    nc = tc.nc
    hc = h // 128          # 4 h-chunks
    zc = 4 * hc            # 16 chunks of the 4H interaction dim
    gb = 2                 # batches per group (matmul moving dim = gb*s_len)
    ng = b_loc // gb
    sc_ = s_len // 128

    facts, questions, prevM = io["facts"], io["questions"], io["prevM"]

    with ExitStack() as ctx:
        # ---------------- resident pools ----------------
        wpool = ctx.enter_context(tc.tile_pool(name="wres", bufs=1))
        prepool = ctx.enter_context(tc.tile_pool(name="prepool", bufs=1))
        smallpool = ctx.enter_context(tc.tile_pool(name="small", bufs=1))
        pfpool = ctx.enter_context(tc.tile_pool(name="pf", bufs=1))

        # prefetch the first 2-batch group of facts ahead of the weight DMAs
        # so the PE can start transposing ~40us earlier
        fpre = pfpool.tile([128, gb * sc_, h], F32, tag="fpre")
        for bp in range(gb):
            for sh in range(sc_):
                nc.sync.dma_start(
                    fpre[:, bp * sc_ + sh, :],
                    facts[bp, sh * 128:(sh + 1) * 128, :],
                )

        # scan gate weights [Ur | U]: k-chunk c at cols [c*2h, (c+1)*2h)
        # (DMAs are emitted in P5 so they don't delay the P1/P2 startup)
        wcomb = wpool.tile([128, hc * 2 * h], F32R, tag="wcomb")

        # small constants: (128, hc) with col = h-chunk
        def load_cvec(nm):
            t = smallpool.tile([128, hc], F32, tag=f"cv_{nm}")
            nc.sync.dma_start(t[:, :], io[nm].rearrange("(c p) -> p c", p=128))
            return t

        z1b4 = load_cvec("z1_b")
        br4 = load_cvec("br")
        bur4 = load_cvec("bur")
        bw4 = load_cvec("bw")
        z2c = smallpool.tile([128, hc], BF16, tag="z2c")
        z2stg = smallpool.tile([128, hc], F32, tag="z2stg")
        nc.sync.dma_start(
            z2stg[:, :], io["z2_w"].rearrange("(c p) o -> p (c o)", p=128)
        )
        nc.vector.tensor_copy(z2c[:, :], z2stg[:, :])
        brc4 = smallpool.tile([128, hc], F32, tag="brc4")  # br + bur
        nc.vector.tensor_copy(brc4[:, :], br4[:, :])
        nc.vector.tensor_add(brc4[:, :], brc4[:, :], bur4[:, :])

        # bu as a row [1, h] (bf16) + ones row for psu PSUM bias init
        bu_stg = smallpool.tile([1, h], F32, tag="bu_stg")
        nc.sync.dma_start(bu_stg[:, :], io["bu"][None, :])
        bu_row = smallpool.tile([1, h], BF16, tag="bu_row")
        nc.vector.tensor_copy(bu_row[:, :], bu_stg[:, :])
        onesb_stg = smallpool.tile([1, b_loc], F32, tag="onesb_stg")
        nc.vector.memset(onesb_stg[:, :], 1.0)
        onesb = smallpool.tile([1, b_loc], BF16, tag="onesb")
        nc.vector.tensor_copy(onesb[:, :], onesb_stg[:, :])

        # questions / prevM transposed, b-major free layout (128, b_loc, hc)
        # so the gather merges into one DMA descriptor each; the dma_start
        # calls are emitted after the z1 weight load (they'd block the sync
        # queue ~6us each otherwise)
        qT = smallpool.tile([128, b_loc, hc], F32R, tag="qT")
        mT = smallpool.tile([128, b_loc, hc], F32R, tag="mT")
        nqT = smallpool.tile([128, b_loc, hc], F32, tag="nqT")
        nmT = smallpool.tile([128, b_loc, hc], F32, tag="nmT")

        ones_row = smallpool.tile([1, b_loc], F32R, tag="ones_row")
        nc.vector.tensor_copy(ones_row[:, :], onesb_stg[:, :])
        nmb_row = smallpool.tile([1, h], F32R, tag="nmb_row")
        nc.sync.dma_start(nmb_row[:, :], io["nm_b"][None, :].bitcast(F32R))

        ident = smallpool.tile([128, 128], F32, tag="ident")
        masks.make_identity(nc, ident[:, :])

        # pre-activations resident through the scan: [p, gate, s, m, b] bf16
        # (s-major so the per-step slice [m, b] is contiguous)
        pre_sb = prepool.tile([128, 2, s_len, hc, b_loc], BF16, tag="pre_sb")
        logit = smallpool.tile([b_loc, s_len], F32, tag="logit")

        # ============ phases P1..P3 (per 2-batch group) ============
        with (
            tc.tile_pool(name="phw", bufs=1) as phw,
            tc.tile_pool(name="ph", bufs=2) as ph,
            tc.tile_pool(name="zpool", bufs=3) as zp,
            tc.tile_pool(name="ghpool", bufs=1) as ghpool,
            tc.tile_pool(name="tps", bufs=2, space="PSUM") as tps,
            tc.tile_pool(name="ghps", bufs=1, space="PSUM") as ghps,
            tc.tile_pool(name="lgps", bufs=1, space="PSUM") as lgps,
        ):
            # z1 weights in fp8e4 (scaled by Z1SC; staged through f32),
            # laid out [128, k-tile, h] for DoubleRow matmuls. One big DMA +
            # one big cast — a chunked DMA/cast pipeline here stalls the
            # whole P2 startup on staging-buffer reuse.
            # z1 weights: 2 chunked DMA+cast stages (the first DoubleRow
            # matmuls need only the early k-tiles); q/m gathers (slow
            # 4B-element DMAs) go after the weights
            z1stg = phw.tile([128, zc, h], F32, tag="z1stg")
            z1w = phw.tile([128, zc, h], FP8, tag="z1w")
            zw_src = io["z1_w"].rearrange("(k p) h -> p k h", p=128)
            for ch in range(2):
                kk = slice(8 * ch, 8 * ch + 8)
                nc.sync.dma_start(z1stg[:, kk, :], zw_src[:, kk, :])
                nc.vector.tensor_scalar_mul(
                    z1w[:, kk, :], z1stg[:, kk, :], Z1SC
                )
            nc.sync.dma_start(
                qT[:, :, :],
                questions[:, 0, :].rearrange(
                    "b (c p) -> p b c", p=128).bitcast(F32R),
            )
            nc.sync.dma_start(
                mT[:, :, :],
                prevM[:, 0, :].rearrange(
                    "b (c p) -> p b c", p=128).bitcast(F32R),
            )
            nc.vector.tensor_scalar_mul(
                nqT[:, :, :], qT[:, :, :].bitcast(F32), -1.0
            )
            nc.vector.tensor_scalar_mul(
                nmT[:, :, :], mT[:, :, :].bitcast(F32), -1.0
            )
            # row-sums of Wr / W (the reference's einsum "bsh,hk->bsh"
            # multiplies facts elementwise by these row-sums)
            rs4 = phw.tile([128, hc, 2], F32, tag="rs4")
            for gate, wname in enumerate(["Wr", "W"]):
                wstg = phw.tile([128, hc, h], F32, tag=f"wstg{gate}")
                nc.sync.dma_start(
                    wstg[:, :, :],
                    io[wname].rearrange("(c p) h -> p c h", p=128),
                )
                for c in range(hc):
                    nc.vector.tensor_reduce(
                        rs4[:, c, gate:gate + 1], wstg[:, c, :],
                        mybir.AxisListType.X, ALU.add,
                    )

            for g in range(ng):
                # ---- P1: load + transpose facts ----
                fT2 = ph.tile([128, hc, gb * s_len], BF16, tag="fT2")
                for bp in range(gb):
                    bi = g * gb + bp
                    for sh in range(sc_):
                        if g == 0:
                            fnat = fpre[:, bp * sc_ + sh, :]
                        else:
                            fnat = ph.tile([128, h], F32, tag="fnat")
                            nc.sync.dma_start(
                                fnat[:, :], facts[bi, sh * 128:(sh + 1) * 128, :]
                            )
                        tp = tps.tile([128, hc, 128], F32, tag="tpsum")
                        for c in range(hc):
                            nc.tensor.transpose(
                                tp[:, c, :], fnat[:, c * 128:(c + 1) * 128],
                                ident[:, :],
                            )
                        nc.vector.tensor_copy(
                            fT2[:, :, bp * s_len + sh * 128:
                                bp * s_len + (sh + 1) * 128],
                            tp[:, :, :],
                        )

                # ---- P2: interaction features (fp8) + z1 MLP (DoubleRow)
                #      + z2 logits ----
                ghp = [ghps.tile([128, gb * s_len], F32, name=f"ghp{m}",
                                 tag=f"ghp{m}") for m in range(hc)]
                for j in range(zc // 2):
                    zk2 = zp.tile([128, 2, gb * s_len], FP8, tag="zk")
                    for half in range(2):
                        k = 2 * j + half
                        kind, c = divmod(k, hc)  # 0:f*q 1:f*m 2:|f-q| 3:|f-m|
                        for bp in range(gb):
                            bi = g * gb + bp
                            dst = zk2[:, half, bp * s_len:(bp + 1) * s_len]
                            src = fT2[:, c, bp * s_len:(bp + 1) * s_len]
                            if kind == 0:
                                nc.vector.tensor_scalar_mul(
                                    dst, src, qT[:, bi, c:c + 1].bitcast(F32))
                            elif kind == 1:
                                nc.vector.tensor_scalar_mul(
                                    dst, src, mT[:, bi, c:c + 1].bitcast(F32))
                            elif kind == 2:
                                nc.scalar.activation(dst, src, AF.Abs,
                                                     bias=nqT[:, bi, c:c + 1])
                            else:
                                nc.scalar.activation(dst, src, AF.Abs,
                                                     bias=nmT[:, bi, c:c + 1])
                    for m in range(hc):
                        nc.tensor.matmul(
                            ghp[m][:, :],
                            z1w[:, 2 * j:2 * j + 2, m * 128:(m + 1) * 128],
                            zk2[:, :, :],
                            start=(j == 0),
                            stop=(j == zc // 2 - 1),
                            perf_mode=DR,
                        )
                ghT = ph.tile([128, hc, gb * s_len], BF16, tag="ghT")
                for m in range(hc):
                    nc.scalar.activation(
                        ghT[:, m, :], ghp[m][:, :], AF.Tanh,
                        bias=z1b4[:, m:m + 1], scale=1.0 / Z1SC,
                    )
                lgp = lgps.tile([1, gb * s_len], F32, tag="lgp")
                for m in range(hc):
                    nc.tensor.matmul(
                        lgp[:, :], z2c[:, m:m + 1], ghT[:, m, :],
                        start=(m == 0), stop=(m == hc - 1),
                    )
                lstage = ph.tile([1, gb * s_len], F32, tag="lstage")
                nc.vector.tensor_copy(lstage[:, :], lgp[:, :])
                nc.sync.dma_start(
                    io["logit_dram"][None, g * gb:(g + 1) * gb, :],
                    lstage[:, :].rearrange("o (b s) -> o b s", b=gb),
                )

                # ---- P3: pre_r / pre_h = facts * rowsum(W) + bias ----
                for gate in range(2):
                    bias4 = brc4 if gate == 0 else bw4
                    for m in range(hc):
                        nc.vector.tensor_scalar(
                            pre_sb[:, gate, :, m, g * gb:(g + 1) * gb],
                            fT2[:, m, :].rearrange("p (b s) -> p s b", b=gb),
                            rs4[:, m, gate:gate + 1],
                            bias4[:, m:m + 1],
                            ALU.mult,
                            ALU.add,
                        )

        # ============ P4: softmax over S + blend-weight broadcast ============
        # The scan processes blocks of BLK steps with C frozen per block; the
        # within-block state update telescopes exactly:
        #   D_BLK = sum_t w_t * e_t,  w_t = g_t * prod_{u>t} (1 - g_u)
        # Compute w in (b, s) layout, then broadcast to all partitions.
        BLK = 8
        abw = smallpool.tile([128, s_len + s_len // BLK, 1, b_loc], F32, tag="abw")
        abc = abw[:, :s_len, :, :]     # w_t broadcast
        wsc = abw[:, s_len:, :, :]     # per-block sum_t w_t broadcast
        with tc.tile_pool(name="smax", bufs=1) as sp:
            nc.sync.dma_start(logit[:, :], io["logit_dram"][:, :])
            negmax = sp.tile([b_loc, 1], F32, tag="negmax")
            nc.vector.tensor_reduce(
                negmax[:, :], logit[:, :], mybir.AxisListType.X, ALU.max, negate=True
            )
            esum = sp.tile([b_loc, 1], F32, tag="esum")
            gexp = sp.tile([b_loc, s_len], F32, tag="gexp")
            nc.scalar.activation(
                gexp[:, :], logit[:, :], AF.Exp, bias=negmax[:, :],
                accum_out=esum[:, :],
            )
            inv = sp.tile([b_loc, 1], F32, tag="inv")
            nc.vector.reciprocal(inv[:, :], esum[:, :])
            gmat = sp.tile([b_loc, s_len], F32, tag="gmat")
            nc.vector.tensor_scalar_mul(gmat[:, :], gexp[:, :], inv[:, :])

            # blend weights w_t = g_t * prod_{u>t in blk} (1-g_u) via suffix
            # products within each BLK-sized block
            om = sp.tile([b_loc, s_len], F32, tag="om")
            nc.vector.tensor_scalar(
                om[:, :], gmat[:, :], -1.0, 1.0, ALU.mult, ALU.add
            )
            wmat = sp.tile([b_loc, s_len], F32, tag="wmat")
            nbk = s_len // BLK
            nc.vector.tensor_copy(wmat[:, BLK - 1::BLK], gmat[:, BLK - 1::BLK])
            suf = sp.tile([b_loc, nbk], F32, tag="suf_a")
            nc.vector.tensor_copy(suf[:, :], om[:, BLK - 1::BLK])
            for t in range(BLK - 2, -1, -1):
                nc.vector.tensor_tensor(
                    wmat[:, t::BLK], gmat[:, t::BLK], suf[:, :], ALU.mult
                )
                if t > 0:
                    suf2 = sp.tile(
                        [b_loc, nbk], F32,
                        tag="suf_b" if (BLK - t) % 2 == 0 else "suf_a",
                    )
                    nc.vector.tensor_tensor(
                        suf2[:, :], om[:, t::BLK], suf[:, :], ALU.mult
                    )
                    suf = suf2
            # per-block sum of the w_t (pairwise reduce)
            cur = wmat
            width = s_len
            lvl = 0
            while width > nbk:
                nxt = sp.tile([b_loc, width // 2], F32, tag=f"ws{lvl}")
                nc.vector.tensor_tensor(
                    nxt[:, :], cur[:, 0::2], cur[:, 1::2], ALU.add
                )
                cur = nxt
                width //= 2
                lvl += 1

            # broadcast w + wsum to all partitions through a DRAM bounce
            nc.sync.dma_start(
                io["g_bounce"][:s_len, :].rearrange("s b -> b s"), wmat[:, :]
            )
            nc.sync.dma_start(
                io["g_bounce"][s_len:, :].rearrange("s b -> b s"), cur[:, :]
            )
            nc.sync.dma_start(
                abw[:, :, 0, :],
                io["g_bounce"][None, :, :].to_broadcast(
                    [128, s_len + s_len // BLK, b_loc]
                ),
            )

        # ============ P5: GRU scan (blocked steps, delta-PSUM form) ============
        # Steps are processed in blocks of BLK. The gate pre-activations for
        # block k use the state snapshot taken after block k-1's matmuls,
        # which itself lags one block (two-block-stale gates, <= 2*BLK-1
        # steps; g ~ 1/256 keeps the error ~1e-3 relative). This decouples
        # the PE pass and the PSUM->SBUF snapshot from the serial DVE/ACT
        # chain: they hide inside the sigmoid/tanh gaps. The within-block
        # state update is EXACT via the telescoped form
        #   D = sum_t w_t h_t - (sum_t w_t) C_s0.
        # psr/psu live in PSUM the whole scan and accumulate W^T @ D once per
        # block.
        n_blk = s_len // BLK
        with (
            tc.tile_pool(name="scw", bufs=1) as scw,
            tc.tile_pool(name="scan_sb", bufs=2) as scp,
            tc.tile_pool(name="scan_ps", bufs=1, space="PSUM") as sps,
            tc.tile_pool(name="out_ps", bufs=1, space="PSUM") as ops,
        ):
            # load + cast the scan weights to bf16 (1 cycle/row on the PE vs
            # 4 for f32r at narrow moving); DMAs deferred to here so the
            # P1/P2 startup isn't delayed
            for c in range(hc):
                nc.sync.dma_start(
                    wcomb[:, c * 2 * h: c * 2 * h + h],
                    io["Ur"][c * 128:(c + 1) * 128, :].bitcast(F32R),
                )
                nc.sync.dma_start(
                    wcomb[:, c * 2 * h + h: (c + 1) * 2 * h],
                    io["U"][c * 128:(c + 1) * 128, :].bitcast(F32R),
                )
            wb = scw.tile([128, hc * 2 * h], BF16, tag="wb")
            nc.vector.tensor_copy(wb[:, :], wcomb[:, :].bitcast(F32))

            # final-layer weights (loaded while the scan runs)
            nmw = scw.tile([128, 3 * hc * h], F32R, tag="nmw")
            for j in range(3 * hc):
                nc.sync.dma_start(
                    nmw[:, j * h:(j + 1) * h],
                    io["nm_w"][j * 128:(j + 1) * 128, :].bitcast(F32R),
                )

            # persistent PSUM accumulators: psr/psu = W^T @ C_{2k} (+ bu)
            psr = sps.tile([128, 1, hc, b_loc], F32, tag="psr")
            psu = sps.tile([128, 1, hc, b_loc], F32, tag="psu")
            # psu init = bu (broadcast over batch) via K=1 matmuls; psr gets
            # start=True on its first real matmul (pair k=1)
            for m in range(hc):
                nc.tensor.matmul(
                    psu[:, 0, m, :], bu_row[:, m * 128:(m + 1) * 128],
                    onesb[:, :], start=True, stop=False, skip_group_check=True,
                )

            ct = None     # C_{s0} (f32), set by the first tail
            wc = None     # (sum w)*C snapshot for the gsum of the next block
            bck = [128, BLK, hc, b_loc]
            bc1 = [128, 1, hc, b_loc]

            # bu broadcast for the first two blocks' gate math
            bu4 = load_cvec("bu")
            bu_bc = scw.tile(bc1, F32, tag="bu_bc")
            ones_pb = scw.tile([128, b_loc], F32, tag="ones_pb")
            nc.vector.memset(ones_pb[:, :], 1.0)
            for c in range(hc):
                nc.vector.tensor_scalar_mul(
                    bu_bc[:, 0, c, :], ones_pb[:, :], bu4[:, c:c + 1]
                )

            def chain(k, psrS, psuS):
                """gate math for block k using (stale) state snapshots;
                emits the ct/wc updates of block k-1 into the sigmoid gap."""
                s0 = BLK * k
                if psrS is None:
                    rtp = scp.tile(bck, BF16, tag="rtp")
                    nc.scalar.activation(
                        rtp[:, :, :, :], pre_sb[:, 0, s0:s0 + BLK, :, :],
                        AF.Sigmoid,
                    )
                else:
                    sgp = scp.tile(bck, BF16, tag="sgp")
                    nc.vector.tensor_tensor(
                        sgp[:, :, :, :],
                        psrS[:, 0:1, :, :].to_broadcast(bck),
                        pre_sb[:, 0, s0:s0 + BLK, :, :], ALU.add,
                    )
                    rtp = scp.tile(bck, BF16, tag="rtp")
                    nc.scalar.activation(
                        rtp[:, :, :, :], sgp[:, :, :, :], AF.Sigmoid
                    )
                # ct/wc updates of the previous block run during the sigmoid
                emit_state_update(k - 1)
                ut2p = scp.tile(bck, BF16, tag="ut2p")
                nc.vector.tensor_tensor(
                    ut2p[:, :, :, :], rtp[:, :, :, :],
                    psuS[:, 0:1, :, :].to_broadcast(bck), ALU.mult,
                )
                hinp = scp.tile(bck, BF16, tag="hinp")
                nc.vector.tensor_tensor(
                    hinp[:, :, :, :], ut2p[:, :, :, :],
                    pre_sb[:, 1, s0:s0 + BLK, :, :], ALU.add,
                )
                htp = scp.tile(bck, BF16, tag="htp")
                nc.scalar.activation(htp[:, :, :, :], hinp[:, :, :, :], AF.Tanh)
                return htp

            # rsum (+gsum) of each block, kept so the state update of block k
            # can be emitted later (it runs inside block k+1's sigmoid gap)
            rsums = {}
            gsums = {}

            def blk_tail(htp, k):
                """telescoped state delta of block k:
                D = sum_t w_t h_t - (sum_t w_t) C_{s0}"""
                s0 = BLK * k
                xh = scp.tile(bck, BF16, tag="xh")
                nc.vector.tensor_tensor(
                    xh[:, :, :, :], htp[:, :, :, :],
                    abc[:, s0:s0 + BLK, :, :].to_broadcast(bck), ALU.mult,
                )
                half = BLK // 2
                cur = xh
                width = BLK
                lvl = 0
                while width > 2:
                    nxt = scp.tile([128, width // 2, hc, b_loc], F32,
                                   tag=f"red{lvl}")
                    nc.vector.tensor_add(
                        nxt[:, :, :, :], cur[:, 0:width // 2, :, :],
                        cur[:, width // 2:width, :, :],
                    )
                    cur = nxt
                    width //= 2
                    lvl += 1
                if k == 0:
                    # C_0 = 0: D = rsum directly
                    gsum = scp.tile([128, hc, b_loc], BF16, tag="gsum")
                    nc.vector.tensor_add(
                        gsum[:, :, :], cur[:, 0, :, :], cur[:, 1, :, :]
                    )
                    gsums[k] = gsum
                else:
                    rsum = scp.tile([128, hc, b_loc], F32, tag="rsum")
                    nc.vector.tensor_add(
                        rsum[:, :, :], cur[:, 0, :, :], cur[:, 1, :, :]
                    )
                    gsum = scp.tile([128, hc, b_loc], BF16, tag="gsum")
                    nc.vector.tensor_sub(
                        gsum[:, :, :], rsum[:, :, :], wc[:, 0, :, :]
                    )
                    gsums[k] = gsum
                return gsums[k]

            def emit_state_update(k):
                """ct/wc update for block k (depends on gsum_k); emitted one
                block later so it runs inside the sigmoid gap."""
                nonlocal ct, wc
                if k < 0 or k not in gsums:
                    return
                gsum = gsums.pop(k)
                ct_new = scp.tile(bc1, F32, tag="ct")
                if ct is None:
                    nc.gpsimd.tensor_copy(ct_new[:, 0, :, :], gsum[:, :, :])
                else:
                    nc.gpsimd.tensor_add(
                        ct_new[:, 0, :, :], ct[:, 0, :, :], gsum[:, :, :]
                    )
                ct = ct_new
                if k + 1 < n_blk:
                    wc_new = scp.tile(bc1, F32, tag="wc")
                    nc.gpsimd.tensor_tensor(
                        wc_new[:, 0, :, :], ct[:, 0, :, :],
                        wsc[:, k + 1:k + 2, 0, :].to_broadcast(
                            [128, hc, b_loc]
                        ),
                        ALU.mult,
                    )
                    wc = wc_new

            # ---- block 0 and 1: gates use C_0 = 0 (psr=0, psu=bu) ----
            htp = chain(0, None, bu_bc)
            gdp = blk_tail(htp, 0)

            psrS_prev = None
            psuS_prev = None
            for k in range(1, n_blk):
                last = k == n_blk - 1
                # PE: accumulate W^T @ D_{k-1} into psr then psu
                for gate in range(2):
                    ps = psr if gate == 0 else psu
                    for m in range(hc):
                        for c in range(hc):
                            nc.tensor.matmul(
                                ps[:, 0, m, :],
                                wb[:, c * 2 * h + gate * h + m * 128:
                                   c * 2 * h + gate * h + (m + 1) * 128],
                                gdp[:, c, :],
                                start=(gate == 0 and k == 1 and c == 0),
                                stop=(last and c == hc - 1),
                                skip_group_check=True,
                            )
                if k == 1:
                    # block 1 gates also use C_0 (snapshots not ready yet)
                    htp = chain(1, None, bu_bc)
                else:
                    htp = chain(k, psrS_prev, psuS_prev)
                # snapshot psr/psu after this block's matmuls, for block k+1's
                # gates (runs inside the tanh gap on the DVE)
                if not last:
                    psrS = scp.tile(bc1, BF16, tag="psrS")
                    nc.vector.tensor_copy(psrS[:, :, :, :], psr[:, :, :, :])
                    psuS = scp.tile(bc1, BF16, tag="psuS")
                    nc.vector.tensor_copy(psuS[:, :, :, :], psu[:, :, :, :])
                    psrS_prev, psuS_prev = psrS, psuS
                gdp = blk_tail(htp, k)

            # final C = C_{S} (flush the last two state updates)
            emit_state_update(n_blk - 2)
            emit_state_update(n_blk - 1)
            cfin = scp.tile([128, hc, b_loc], F32R, tag="cfin")
            nc.vector.tensor_copy(cfin[:, :, :], ct[:, 0, :, :])

            # ============ P6: next memory ============
            po = ops.tile([b_loc, h], F32, tag="po")
            # mT/qT are b-major [128, b, c]; cfin is [128, c, b]
            chunks = [(mT, True), (cfin, False), (qT, True)]
            for part, (src, bmajor) in enumerate(chunks):
                for c in range(hc):
                    j = part * hc + c
                    nc.tensor.matmul(
                        po[:, :],
                        src[:, :, c] if bmajor else src[:, c, :],
                        nmw[:, j * h:(j + 1) * h],
                        start=(j == 0), stop=False,
                    )
            nc.tensor.matmul(
                po[:, :], ones_row[:, :], nmb_row[:, :], start=False, stop=True
            )
            out_sb = scp.tile([b_loc, h], F32, tag="out_sb")
            nc.scalar.activation(out_sb[:, :], po[:, :], AF.Relu)
            nc.sync.dma_start(io["out"][:, 0, :], out_sb[:, :])


_NC_CACHE = {}


def _run(inputs, **spmd_kwargs):
    if "full" not in _NC_CACHE:
        _NC_CACHE["full"] = build_nc()
    nc = _NC_CACHE["full"]

    names = ["facts", "questions", "prevM", "z1_w", "z1_b", "z2_w",
             "Wr", "br", "Ur", "bur", "W", "bw", "U", "bu", "nm_w", "nm_b"]
    sharded = {"facts", "questions", "prevM"}
    in_maps = []
    for i in range(N_CORES):
        m = {}
        for n in names:
            v = np.asarray(inputs[n], dtype=np.float32)
            if n in sharded:
                v = v[i * B_LOC:(i + 1) * B_LOC]
            m[n] = np.ascontiguousarray(v)
        in_maps.append(m)

    res = run_bass_kernel_spmd(nc, in_maps, list(range(N_CORES)), **spmd_kwargs)
    out = np.concatenate(
        [res.results[i]["out"] for i in range(N_CORES)], axis=0
    ).astype(np.float32)
    return out, res


def kernel(**inputs):
    return _run(inputs)[0]
